# revision 1
# baseline (speedup 1.0000x reference)
"""Contrastive-loss (InfoNCE re-rank) Trainium2 Bass kernel.

Full op: q,k [256,1024], neg [256,2048,1024] f32.
  l_pos[n]   = q[n].k[n]
  l_neg[n,j] = neg[n,j].q[n]
  loss = mean_n( LSE(logits_n/T) - l_pos[n]/T ),  T = 0.07

Sharding: data-parallel over N across 8 NeuronCores (32 samples/core).

Per-core layout: samples are processed in 8 groups of 4. Within a group
the 128 SBUF partitions carry (n4=4 samples x ji=32 j-lanes); each lane
owns jo=64 consecutive negatives, so a partition's DMA line for a
jo-chunk is fully contiguous in HBM (jch*4KB per descriptor; 32KB is
the measured per-SDMA-engine sweet spot). The per-group q broadcast
[128, C] is built on the idle PE as sel.T @ q4 (sel a 0/1 expansion
matrix) directly into PSUM — no HBM broadcast reads. DVE
scalar_tensor_tensor fuses scale*neg*q with the free-dim reduction;
level-1 LSE runs in-stream per group, level-2 crosses partitions via a
PE transpose at the end.
"""

import numpy as np

import concourse.bass as bass
import concourse.bacc as bacc
import concourse.tile as tile
from concourse import mybir
from concourse.masks import make_identity
from concourse.bass_utils import run_bass_kernel_spmd

N, C, K = 256, 1024, 2048
NCORES = 8
NLOC = N // NCORES          # 32 samples per core
P = 128                     # SBUF partitions
G = 8                       # sample groups per core
N4 = NLOC // G              # 4 samples per group
JI = P // N4                # 32 j-lanes per sample
JO = K // JI                # 64 logits per lane (free axis)
# jo-chunk sizes per group: 8 in steady state (4 MB chunks, 32 KB HBM
# descriptors). Small chunks at the very start (shorter pipeline ramp)
# and very end (shorter DVE drain after the last DMA byte).
CHUNKS_FIRST = [2, 2, 4] + [8] * 7
CHUNKS_MID = [8] * 8
CHUNKS_LAST = [8] * 7 + [4, 2, 2]
MM_N = 512                  # PE moving-operand free-dim limit
TEMP = 0.07
SCALE = 1.0 / TEMP
F32 = mybir.dt.float32
ALU = mybir.AluOpType
ACT = mybir.ActivationFunctionType


def build_module() -> bass.Bass:
    # Bacc (not plain Bass): its compile() runs generate_event_semaphores,
    # which splits multi-sem waits into separate event instructions — this
    # walrus rejects >1 sync wait per instruction.
    nc = bacc.Bacc("TRN2", target_bir_lowering=False)
    q_d = nc.dram_tensor("q", [NLOC, C], F32, kind="ExternalInput")
    k_d = nc.dram_tensor("k", [NLOC, C], F32, kind="ExternalInput")
    neg_d = nc.dram_tensor("neg", [NLOC, K, C], F32, kind="ExternalInput")
    out_d = nc.dram_tensor("nll", [G, N4], F32, kind="ExternalOutput")

    # neg[g*4+f, ji*64+jo, c] viewed as [g, (f ji), jo, c]: partition
    # p = f*32+ji, and a jo-chunk is jch contiguous 4KB rows per
    # partition — one jch*4KB descriptor per partition per dma_start.
    neg_v = neg_d[:].rearrange("(g f) (j t) c -> g (f j) t c", f=N4, j=JI)

    with tile.TileContext(nc) as tc:
        with (
            tc.tile_pool(name="consts", bufs=1) as consts,
            tc.tile_pool(name="small", bufs=1) as small,
            tc.tile_pool(name="scr", bufs=2) as scr_pool,
            tc.tile_pool(name="negp", bufs=4) as negp,
            tc.tile_pool(name="ps", bufs=1, space="PSUM") as ps_pool,
            tc.tile_pool(name="qgp", bufs=2, space="PSUM") as qg_pool,
        ):
            identity = consts.tile([P, P], F32)
            make_identity(nc, identity)

            # q as [f, g, c] so every group's 4-row slice starts at
            # partition 0 (matmul base-partition requirement).
            q_sb = consts.tile([N4, G, C], F32)
            nc.sync.dma_start(out=q_sb, in_=q_d[:].rearrange("(g f) c -> f g c", f=N4))

            # sel[f, p] = 1 iff p//32 == f: qg = sel.T @ q4 replicates each
            # of the group's 4 q rows across its 32 partition lanes. Built
            # transposed (engine APs need partition base 0/32/64/96), then
            # flipped on the PE.
            sel_t = consts.tile([P, N4], F32)
            nc.vector.memset(sel_t, 0.0)
            for f in range(N4):
                nc.vector.memset(sel_t[f * JI : (f + 1) * JI, f : f + 1], 1.0)
            ps_sel = ps_pool.tile([N4, P], F32)
            nc.tensor.transpose(out=ps_sel, in_=sel_t, identity=identity)
            sel = consts.tile([N4, P], F32)
            nc.scalar.copy(out=sel, in_=ps_sel)

            qgs = {}

            def build_qg(g):
                qg = qg_pool.tile([P, C], F32, tag="qg")
                for h in range(C // MM_N):
                    nc.tensor.matmul(
                        out=qg[:, h * MM_N : (h + 1) * MM_N],
                        lhsT=sel,
                        rhs=q_sb[:, g, h * MM_N : (h + 1) * MM_N],
                        start=True,
                        stop=True,
                    )
                qgs[g] = qg

            build_qg(0)

            # q/k as [g, (f c)] for the positive logits.
            q8 = consts.tile([G, N4 * C], F32)
            k8 = consts.tile([G, N4 * C], F32)
            nc.sync.dma_start(out=q8, in_=q_d[:].rearrange("(g f) c -> g (f c)", f=N4))
            nc.sync.dma_start(out=k8, in_=k_d[:].rearrange("(g f) c -> g (f c)", f=N4))

            # ypos[g, f] = (q[n].k[n]) / T for n = g*4+f.
            ypos = small.tile([G, N4, 1], F32)
            for f in range(N4):
                pos_scr = scr_pool.tile([P, C], F32, tag="ttr")
                nc.vector.scalar_tensor_tensor(
                    out=pos_scr[0:G, :],
                    in0=q8[:, f * C : (f + 1) * C],
                    scalar=SCALE,
                    in1=k8[:, f * C : (f + 1) * C],
                    op0=ALU.mult,
                    op1=ALU.mult,
                    accum_out=ypos[:, f, :],
                )

            # Scaled negative logits Y[p, g, j], plus in-stream level-1 LSE
            # (per-group max + exp-sum overlap the next group's DMA).
            Y = small.tile([P, G, JO], F32)
            m1 = small.tile([P, G], F32)
            m1neg = small.tile([P, G], F32)
            s1 = small.tile([P, G], F32)
            for g in range(G):
                sched = (
                    CHUNKS_FIRST if g == 0
                    else CHUNKS_LAST if g == G - 1
                    else CHUNKS_MID
                )
                if g + 1 < G:
                    build_qg(g + 1)
                jo0 = 0
                for jch in sched:
                    neg_t = negp.tile([P, jch, C], F32, tag="neg")
                    nc.sync.dma_start(
                        out=neg_t,
                        in_=neg_v[g, :, jo0 : jo0 + jch, :],
                    )
                    for t in range(jch):
                        ttr_scr = scr_pool.tile([P, C], F32, tag="ttr")
                        nc.vector.scalar_tensor_tensor(
                            out=ttr_scr,
                            in0=neg_t[:, t, :],
                            scalar=SCALE,
                            in1=qgs[g],
                            op0=ALU.mult,
                            op1=ALU.mult,
                            accum_out=Y[:, g, jo0 + t : jo0 + t + 1],
                        )
                    jo0 += jch
                nc.vector.reduce_max(
                    out=m1[:, g : g + 1], in_=Y[:, g, :], axis=mybir.AxisListType.X
                )
                nc.scalar.mul(m1neg[:, g : g + 1], m1[:, g : g + 1], -1.0)
                e_scr = scr_pool.tile([P, JO], F32, tag="esc")
                nc.scalar.activation(
                    out=e_scr, in_=Y[:, g, :], func=ACT.Exp,
                    bias=m1neg[:, g : g + 1], scale=1.0,
                    accum_out=s1[:, g : g + 1],
                )

            lse1 = small.tile([P, G], F32)
            nc.scalar.activation(out=lse1, in_=s1, func=ACT.Ln)
            nc.vector.tensor_add(out=lse1, in0=lse1, in1=m1)

            # Level-2 LSE across partitions: transpose so groups sit on
            # partitions; sample n = g*4+f owns pt[g, f*32:(f+1)*32].
            pt = ps_pool.tile([G, P], F32)
            nc.tensor.transpose(out=pt, in_=lse1, identity=identity)
            ltf = small.tile([G, N4, JI + 1], F32)
            nc.scalar.copy(
                out=ltf[:, :, 0:JI],
                in_=pt[:].rearrange("g (f j) -> g f j", f=N4),
            )
            nc.vector.tensor_copy(out=ltf[:, :, JI : JI + 1], in_=ypos)

            m2 = small.tile([G, N4], F32)
            nc.vector.reduce_max(out=m2, in_=ltf, axis=mybir.AxisListType.X)
            m2neg = small.tile([G, N4], F32)
            nc.scalar.mul(m2neg, m2, -1.0)
            s2 = small.tile([G, N4], F32)
            for f in range(N4):
                e2_scr = scr_pool.tile([G, JI + 1], F32, tag="e2")
                nc.scalar.activation(
                    out=e2_scr, in_=ltf[:, f, :], func=ACT.Exp,
                    bias=m2neg[:, f : f + 1], scale=1.0,
                    accum_out=s2[:, f : f + 1],
                )
            ln2 = small.tile([G, N4], F32)
            nc.scalar.activation(out=ln2, in_=s2, func=ACT.Ln)
            nc.vector.tensor_add(out=ln2, in0=ln2, in1=m2)

            # nll[g, f] = lse2 - ypos
            nll = small.tile([G, N4], F32)
            nc.vector.tensor_sub(out=nll, in0=ln2, in1=ypos[:, :, 0])
            nc.sync.dma_start(out=out_d[:], in_=nll)

    nc.finalize()
    return nc


_CACHED = {}


def _run(q, k, neg, trace=False):
    if "nc" not in _CACHED:
        _CACHED["nc"] = build_module()
    nc = _CACHED["nc"]
    in_maps = []
    for c in range(NCORES):
        s = slice(c * NLOC, (c + 1) * NLOC)
        in_maps.append({"q": q[s], "k": k[s], "neg": neg[s]})
    res = run_bass_kernel_spmd(
        nc, in_maps, core_ids=list(range(NCORES)), trace=trace
    )
    nll = np.concatenate([r["nll"].reshape(-1) for r in res.results])
    loss = np.asarray(np.mean(nll.astype(np.float64)), dtype=np.float32)
    return loss, res


def kernel(q, k, neg):
    q = np.ascontiguousarray(np.asarray(q, dtype=np.float32))
    k = np.ascontiguousarray(np.asarray(k, dtype=np.float32))
    neg = np.ascontiguousarray(np.asarray(neg, dtype=np.float32))
    loss, _ = _run(q, k, neg, trace=False)
    return loss

